# revision 31
# baseline (speedup 1.0000x reference)
"""Trainium2 Bass kernel for nn_AcceptHead: out = fc2(gelu(fc1(LN(x)))).

Self-contained: hardcodes shapes (B=4, L=4096, H=4096, F=1024) and the
data-parallel sharding (tokens split 8 ways, head params replicated).

"LN-fold" architecture (v2): the device PE stream is *only* fc1 matmuls.
LayerNorm is folded into the matmul epilogue:

    LN(x)@W = r_t * (x@W) - r_t * mu_t * colsum(W)     (gamma folded into W)

  - x arrives already TRANSPOSED from the host (xts: [128, chunk, k, tok]
    fp16, chunk-blocked) -- no on-device transpose, no normalize pass.
  - fc1: lhsT = xT block [128h x 128t], rhs = w1ts [128h x 512f], PSUM
    accumulation over 32 k-tiles (fp16, the PE floor: ~218us @2.4GHz).
  - mean correction: one K=1 matmul per (t-tile, f-half) adds
    (-mu_t) * colsum_f into the same PSUM group (lhsT = row of -mu at
    partition 0 made by a tiny PE transpose; rhs = colsum row).
  - rstd r_t is applied as the per-partition `scale` AP of the Gelu
    activation: g = Gelu(r * psum). Newton rsqrt on DVE as before.
  - stats (sum / sumsq) come from a second wire copy of x in [tok, H]
    fp16 layout (DVE reduce_sum + ACT Square accum_out) -- off the
    critical path entirely.
  - fc2 as DVE dot (g * w2_bcast, reduce) as before.

DMA placement: three queues in parallel during fill -- SP ring carries
xts + xs chunk streams (split into sub-DMAs so the PE can chase arriving
k-blocks), ACT ring carries w1ts f-half 0, SWDGE (gpsimd) carries w1ts
f-half 1 + small params. HWDGE dma_start blocks the issuing sequencer,
which is fine: ACT has no work until the first stats tiles land.

PSUM: 3 x [128,1024] fc1 accumulators (6 banks) + 2 x [1,256] transpose
landing tiles (2 banks) = 8 banks exactly.

fp8 was investigated and rejected: DoubleRow measures 2x fp16 per unit
contraction on this HW (3826ns vs 7386ns per K=4096,N=512 group; the
cost model's 0.5 cyc/row = 4x is wrong here), and at 2x every
precision-passing fp8 scheme costs the same as fp16 (1-pass e4m3 fails
the 2e-2 gate at 4.2e-2 measured in simulation).
"""

import os
import sys

for _p in ("/opt/trn_rl_repo", "/root/.axon_site/_ro/trn_rl_repo"):
    if os.path.isdir(_p) and _p not in sys.path:
        sys.path.append(_p)

import numpy as np

import concourse.bacc as bacc
import concourse.mybir as mybir
import concourse.tile as tile
from concourse.bass_utils import run_bass_kernel_spmd

N_CORES = 8
B, L, H = 4, 4096, 4096
F = H // 4
F2 = F // 2                   # 512, f-half width
T_TOT = B * L                 # 16384 tokens
T_CORE = T_TOT // N_CORES     # 2048 tokens per core
P = 128
KT = H // P                   # 32 contraction tiles
CHUNK_T = 256                 # tokens per pipeline chunk
N_CHUNKS = T_CORE // CHUNK_T  # 8
TT = CHUNK_T // P             # t-tiles per chunk (2)
N_TTILES = T_CORE // P        # 16
EPS = 1e-5
RSQRT_MAGIC = 0x5F3759DF

F16 = mybir.dt.float16
F32 = mybir.dt.float32
F8 = mybir.dt.float8e4
I32 = mybir.dt.int32
AF = mybir.ActivationFunctionType
ALU = mybir.AluOpType


def build_program(has_bias1: bool, bias2_val: float):
    nc = bacc.Bacc(
        "TRN2",
        target_bir_lowering=False,
        debug=False,
        enable_asserts=False,
        num_devices=N_CORES,
    )
    # x, transposed+chunk-blocked on host: xts[p, c, k, t] = x[c*256+t, k*128+p]
    xts_d = nc.dram_tensor(
        "xts", [P, N_CHUNKS, KT, CHUNK_T], F16, kind="ExternalInput"
    ).ap()
    # x, natural [tok, H] layout (stats only; fp8 halves its wire cost and
    # the ~2.6% quantization only perturbs mu/r by ~4e-4 relative)
    xs_d = nc.dram_tensor("xs", [T_CORE, H], F8, kind="ExternalInput").ap()
    # w1 (gamma-folded, transposed): w1ts[p, fh, k, j] = w1g[k*128+p, fh*512+j]
    w1ts_d = nc.dram_tensor(
        "w1ts", [P, 2, KT, F2], F16, kind="ExternalInput"
    ).ap()
    cs_d = nc.dram_tensor("cs", [1, F], F16, kind="ExternalInput").ap()
    w2b_d = nc.dram_tensor("w2b", [P, F], F16, kind="ExternalInput").ap()
    if has_bias1:
        b1r_d = nc.dram_tensor("b1r", [1, F], F16, kind="ExternalInput").ap()
    # out as [t-tile, partition]: token t = n*128+p lives at out[n, p], so
    # the flattened DRAM tensor IS token order. A direct (n p) -> p n
    # scatter DMA would cost ~10us in 4-byte descriptors.
    out_d = nc.dram_tensor(
        "out", [N_TTILES, P], F32, kind="ExternalOutput"
    ).ap()

    with tile.TileContext(nc) as tc:
        with (
            tc.tile_pool(name="singles", bufs=1) as singles,
            tc.tile_pool(name="xtpool", bufs=3) as xtpool,
            tc.tile_pool(name="xspool", bufs=6) as xspool,
            tc.tile_pool(name="sqscr", bufs=1) as sqscr_pool,
            tc.tile_pool(name="gpool", bufs=2) as gpool,
            tc.tile_pool(name="fc2scr", bufs=1) as fc2scr_pool,
            tc.tile_pool(name="stats", bufs=4) as stats,
            tc.tile_pool(name="nrow", bufs=2) as nrow_pool,
            tc.tile_pool(name="psum", bufs=3, space="PSUM") as psum_pool,
            tc.tile_pool(name="tpsum", bufs=1, space="PSUM") as tpsum_pool,
            tc.tile_pool(name="opsum", bufs=1, space="PSUM") as opsum_pool,
        ):
            # ---- fill-phase DMA schedule (DMA bandwidth is SHARED across
            # the queues at ~350 GB/s aggregate, so what matters is global
            # priority order): stage 1 = xts-c0 + w1ts f-half 0 split three
            # ways (~6MB, feeds psum groups 0-1), stage 2 = f-half 1 on the
            # two HW rings, stage 3 = xs-c0 + chunk 1+. ----
            w1ts_sb = singles.tile([P, 2, KT, F2], F16)

            def w1_dma(eng, fh, ka, kb):
                eng.dma_start(
                    out=w1ts_sb[:, fh, ka:kb, :], in_=w1ts_d[:, fh, ka:kb, :]
                )

            # stage 1: ACT ring carries xts-c0 (emitted via load_chunk below
            # before anything else lands on nc.scalar); SP + SWDGE carry fh0.
            # first blocks are 2-k so the PE's first matmul starts ~0.8us in
            xt0 = xtpool.tile([P, KT, CHUNK_T], F16, tag="xt")
            xt0_blocks = [(0, 2), (16, 18), (2, 4), (18, 20)] + [
                (k0, k0 + 4) for ka in range(4, KT // 2, 4)
                for k0 in (ka, KT // 2 + ka)
            ]
            for ka, kb in xt0_blocks:
                nc.scalar.dma_start(
                    out=xt0[:, ka:kb, :], in_=xts_d[:, 0, ka:kb, :]
                )
            for ka, kb in [(0, 2), (2, 4)] + [
                (k0, k0 + 4) for k0 in range(4, KT // 2, 4)
            ]:
                w1_dma(nc.sync, 0, ka, kb)
            for ka, kb in [(16, 18), (18, 20)] + [
                (k0, k0 + 4) for k0 in range(20, KT, 4)
            ]:
                w1_dma(nc.gpsimd, 0, ka, kb)
            # stage 2: fh1 split ACT/SP
            for k0 in range(0, KT // 2, 4):
                w1_dma(nc.scalar, 1, k0, k0 + 4)
            for k0 in range(KT // 2, KT, 4):
                w1_dma(nc.sync, 1, k0, k0 + 4)
            # stage 3: xs-c0 on ACT, small params on SWDGE
            xss0 = []
            for i in range(TT):
                xs = xspool.tile([P, H], F8, tag="xs")
                nc.scalar.dma_start(out=xs, in_=xs_d[i * P : (i + 1) * P, :])
                xss0.append(xs)
            cs_sb = singles.tile([1, F], F16)
            nc.gpsimd.dma_start(out=cs_sb, in_=cs_d)
            w2b_sb = singles.tile([P, F], F16)
            nc.gpsimd.dma_start(out=w2b_sb, in_=w2b_d)
            if has_bias1:
                b1r_sb = singles.tile([1, F], F16)
                nc.gpsimd.dma_start(out=b1r_sb, in_=b1r_d)
            outcols = singles.tile([P, N_TTILES], F32)
            outrow = singles.tile([N_TTILES, P], F32)
            ident = singles.tile([P, P], F16)
            ident32 = singles.tile([P, P], F32)
            from concourse.masks import make_identity
            make_identity(nc, ident[:])
            make_identity(nc, ident32[:])

            # ---- chunk input loads for chunks 1+ (SP ring) ----
            def load_chunk(c):
                xt = xtpool.tile([P, KT, CHUNK_T], F16, tag="xt")
                for ka in range(0, KT // 2, 4):
                    for k0 in (ka, KT // 2 + ka):
                        nc.sync.dma_start(
                            out=xt[:, k0 : k0 + 4, :],
                            in_=xts_d[:, c, k0 : k0 + 4, :],
                        )
                xss = []
                for i in range(TT):
                    xs = xspool.tile([P, H], F8, tag="xs")
                    row0 = c * CHUNK_T + i * P
                    nc.sync.dma_start(out=xs, in_=xs_d[row0 : row0 + P, :])
                    xss.append(xs)
                return xt, xss

            # ---- stats chain (DVE + ACT), emitted one chunk AHEAD of the
            # chunk's k-loops so the in-order DVE queue never makes a corr
            # matmul wait behind the previous chunk's fc2 work. ----
            def emit_stats(xss):
                sums = stats.tile([P, TT], F32, tag="sums")
                sq = stats.tile([P, TT], F32, tag="sq")
                for i in range(TT):
                    nc.vector.reduce_sum(
                        sums[:, i : i + 1], xss[i], axis=mybir.AxisListType.X
                    )
                    sqs = sqscr_pool.tile([P, H], F16, tag="sqs")
                    nc.scalar.activation(
                        out=sqs, in_=xss[i], func=AF.Square,
                        accum_out=sq[:, i : i + 1],
                    )
                mu = stats.tile([P, TT], F32, tag="mu")
                nc.vector.tensor_scalar_mul(mu, sums, 1.0 / H)
                vv = stats.tile([P, TT], F32, tag="vv")
                # vv = sq/H - mu^2 + eps
                nc.vector.tensor_tensor(out=vv, in0=mu, in1=mu, op=ALU.mult)
                nc.vector.tensor_scalar(
                    out=vv, in0=vv, scalar1=-1.0, scalar2=EPS,
                    op0=ALU.mult, op1=ALU.add,
                )
                nc.vector.tensor_scalar(
                    out=sq, in0=sq, scalar1=1.0 / H, scalar2=None, op0=ALU.mult
                )
                nc.vector.tensor_tensor(out=vv, in0=vv, in1=sq, op=ALU.add)
                # Newton rsqrt: y0 via bit trick, 2 iterations
                y = stats.tile([P, TT], F32, tag="y")
                yi = y[:].bitcast(I32)
                nc.vector.tensor_scalar(
                    out=yi, in0=vv[:].bitcast(I32), scalar1=1, scalar2=None,
                    op0=ALU.arith_shift_right,
                )
                nc.vector.tensor_scalar(
                    out=yi, in0=yi, scalar1=-1, scalar2=RSQRT_MAGIC,
                    op0=ALU.mult, op1=ALU.add,
                )
                h_half = stats.tile([P, TT], F32, tag="h_half")
                nc.vector.tensor_scalar_mul(h_half, vv, 0.5)
                u = stats.tile([P, TT], F32, tag="u")
                for _ in range(2):
                    nc.vector.tensor_tensor(out=u, in0=y, in1=y, op=ALU.mult)
                    nc.vector.tensor_tensor(out=u, in0=u, in1=h_half, op=ALU.mult)
                    nc.vector.tensor_scalar(
                        out=u, in0=u, scalar1=-1.0, scalar2=1.5,
                        op0=ALU.mult, op1=ALU.add,
                    )
                    nc.vector.tensor_tensor(out=y, in0=y, in1=u, op=ALU.mult)
                # nmr16 = -mu as fp16 (the corr-matmul lhsT operand)
                nmr16 = stats.tile([P, TT], F16, tag="nmr16")
                nc.vector.tensor_scalar_mul(nmr16, mu, -1.0)
                if has_bias1:
                    # invr = sqrt(var+eps) = vv * y; bias row b1_eff enters
                    # PSUM as invr_row.T @ b1r so that Gelu's r-scale cancels.
                    invr16 = stats.tile([P, TT], F16, tag="invr16")
                    nc.vector.tensor_tensor(
                        out=invr16, in0=vv, in1=y, op=ALU.mult
                    )
                else:
                    invr16 = None
                return y, nmr16, invr16

            cur = (xt0, xss0)
            nxt = load_chunk(1)
            st_cur = emit_stats(cur[1])

            nr = ir = None
            for c in range(N_CHUNKS):
                xt, xss = cur
                cur = nxt
                y, nmr16, invr16 = st_cur

                # ---- -mu rows to partition 0 via PE transpose ----
                # Emitted at the TOP of the chunk body (before next-chunk
                # stats) for c>=1: nmr16 was computed a full chunk ago, so
                # the PE transpose + DVE copy retire immediately and the
                # copy never sits behind next-chunk reduces in the in-order
                # DVE queue (which would stall the corr matmuls).
                def emit_nmr_rows():
                    tps = tpsum_pool.tile([1, TT, P], F16, tag="tps")
                    for i in range(TT):
                        nc.tensor.transpose(
                            tps[:, i, :], nmr16[:, i : i + 1], ident[:]
                        )
                    nr = nrow_pool.tile([1, TT, P], F16, tag="nr")
                    nc.vector.tensor_copy(out=nr, in_=tps)
                    if has_bias1:
                        tps2 = tpsum_pool.tile([1, TT, P], F16, tag="tps")
                        for i in range(TT):
                            nc.tensor.transpose(
                                tps2[:, i, :], invr16[:, i : i + 1], ident[:]
                            )
                        ir = nrow_pool.tile([1, TT, P], F16, tag="ir")
                        nc.vector.tensor_copy(out=ir, in_=tps2)
                    else:
                        ir = None
                    return nr, ir

                # ---- per t-tile: k-loops, then (corr MM, gelu, fc2)
                # interleaved right behind them so the epilogue of t-tile i
                # hides under t-tile i+1's k-loops and PSUM frees early. ----
                def emit_epilogue(i, g_ps):
                    for fh in range(2):
                        fcols = slice(fh * F2, (fh + 1) * F2)
                        nc.tensor.matmul(
                            g_ps[:, fcols],
                            lhsT=nr[:, i, :],
                            rhs=cs_sb[:, fcols],
                            start=False,
                            stop=(not has_bias1),
                        )
                        if has_bias1:
                            nc.tensor.matmul(
                                g_ps[:, fcols],
                                lhsT=ir[:, i, :],
                                rhs=b1r_sb[:, fcols],
                                start=False,
                                stop=True,
                            )
                    g_sb = gpool.tile([P, F], F16, tag="g_sb")
                    nc.scalar.activation(
                        out=g_sb, in_=g_ps, func=AF.Gelu, scale=y[:, i : i + 1]
                    )
                    fc2s = fc2scr_pool.tile([P, F], F16, tag="fc2s")
                    gi = c * TT + i
                    nc.vector.tensor_tensor(
                        out=fc2s, in0=g_sb, in1=w2b_sb, op=ALU.mult
                    )
                    nc.vector.reduce_sum(
                        outcols[:, gi : gi + 1], fc2s, axis=mybir.AxisListType.X
                    )

                if c > 0:
                    nr, ir = emit_nmr_rows()
                if c + 1 < N_CHUNKS:
                    st_cur = emit_stats(cur[1])

                def emit_group(i, fh, g_ps):
                    fcols = slice(fh * F2, (fh + 1) * F2)
                    # consume k in lo/hi interleave (k0, k16, k1, k17...)
                    # matching the two DMA queues carrying each f-half,
                    # so a slow queue half doesn't stall the fill
                    for kk in range(KT // 2):
                        for k in (kk, KT // 2 + kk):
                            nc.tensor.matmul(
                                g_ps[:, fcols],
                                lhsT=xt[:, k, i * P : (i + 1) * P],
                                rhs=w1ts_sb[:, fh, k, :],
                                start=(kk == 0 and k == 0),
                                stop=False,
                            )

                psums = []
                if c == 0:
                    # chunk 0: fh-major group order — both t-tiles' fh0
                    # groups run on stage-1 cargo before fh1 (stage 2) lands
                    for i in range(TT):
                        g_ps = psum_pool.tile([P, F], F32, tag="g_ps")
                        psums.append(g_ps)
                    for fh in range(2):
                        for i in range(TT):
                            emit_group(i, fh, psums[i])
                else:
                    for i in range(TT):
                        g_ps = psum_pool.tile([P, F], F32, tag="g_ps")
                        for fh in range(2):
                            emit_group(i, fh, g_ps)
                        emit_epilogue(i, g_ps)

                if c == 0:
                    # chunk 0: stats only land ~22us in, so the rows and
                    # epilogues all go after the k-loops
                    nr, ir = emit_nmr_rows()
                    for i in range(TT):
                        emit_epilogue(i, psums[i])

                if c + 2 < N_CHUNKS:
                    nxt = load_chunk(c + 2)

            if bias2_val != 0.0:
                nc.vector.tensor_scalar_add(outcols, outcols, bias2_val)
            # transpose [128, 16] -> [16, 128] on the PE so the out DMA is
            # 16 contiguous 512B rows instead of 2048 4-byte descriptors
            otp = opsum_pool.tile([N_TTILES, P], F32, tag="otp")
            nc.tensor.transpose(otp[:], outcols[:], ident32[:])
            nc.vector.tensor_copy(out=outrow, in_=otp)
            nc.sync.dma_start(out=out_d, in_=outrow)

    nc.compile()
    return nc


def _prep_host(hidden_states, ln_gamma, ln_beta, w1, bias1, w2, bias2):
    """Host-side marshalling: dtype casts, layout transposes, exact (fp64)
    folding of the LN affine params into fc1."""
    g64 = np.asarray(ln_gamma, np.float64)
    b64 = np.asarray(ln_beta, np.float64)
    w1_64 = np.asarray(w1, np.float64)
    w1g = np.ascontiguousarray((w1_64 * g64[None, :]).T).astype(np.float16)
    # [4096, 1024] -> [128, 2, 32, 512]: w1ts[p, fh, k, j] = w1g[k*128+p, fh*512+j]
    w1ts = np.ascontiguousarray(
        w1g.reshape(KT, P, 2, F2).transpose(1, 2, 0, 3)
    )
    # colsum of the fp16-quantized folded weights (consistency with device MM)
    cs = np.ascontiguousarray(
        w1g.astype(np.float64).sum(axis=0).reshape(1, F)
    ).astype(np.float16)
    b1_eff = (np.asarray(bias1, np.float64) + w1_64 @ b64).astype(np.float32)
    b1r = b1_eff.reshape(1, F).astype(np.float16)
    w2b = np.broadcast_to(
        np.asarray(w2, np.float64).reshape(1, F).astype(np.float16), (P, F)
    ).copy()
    bias2_val = float(np.asarray(bias2).reshape(-1)[0])
    x2 = np.ascontiguousarray(
        np.asarray(hidden_states, np.float32).reshape(T_TOT, H)
    ).astype(np.float16)
    return x2, w1ts, cs, b1r, w2b, bias2_val


_CACHE = {}


def _get_program(has_bias1, bias2_val):
    key = (has_bias1, bias2_val)
    if key not in _CACHE:
        _CACHE[key] = build_program(has_bias1, bias2_val)
    return _CACHE[key]


def make_in_maps(inputs):
    x2, w1ts, cs, b1r, w2b, bias2_val = _prep_host(**inputs)
    has_bias1 = bool(np.any(np.asarray(b1r) != 0.0))
    in_maps = []
    for core in range(N_CORES):
        xc = x2[core * T_CORE : (core + 1) * T_CORE]  # [2048, 4096]
        # xts[p, c, k, t] = xc[c*256+t, k*128+p]
        xts = np.ascontiguousarray(
            xc.reshape(N_CHUNKS, CHUNK_T, KT, P).transpose(3, 0, 2, 1)
        )
        import ml_dtypes

        m = {
            "xts": xts,
            "xs": np.ascontiguousarray(xc).astype(ml_dtypes.float8_e4m3),
            "w1ts": w1ts,
            "cs": cs,
            "w2b": w2b,
        }
        if has_bias1:
            m["b1r"] = b1r
        in_maps.append(m)
    return in_maps, has_bias1, bias2_val


def kernel(**inputs) -> np.ndarray:
    in_maps, has_bias1, bias2_val = make_in_maps(inputs)
    nc = _get_program(has_bias1, bias2_val)
    res = run_bass_kernel_spmd(nc, in_maps, core_ids=list(range(N_CORES)))
    out = np.concatenate(
        [np.asarray(res.results[i]["out"]).reshape(-1) for i in range(N_CORES)]
    )
    return out.reshape(B, L).astype(np.float32)
